# revision 59
# baseline (speedup 1.0000x reference)
"""Chamfer distance (CDLoss) Trainium2 Bass kernel.

Problem: B=8, N=4096, D=3.
  T[b,i,j] = ||pred[b,i] - gt[b,j]||^2
  loss = (sum_bj min_i T + sum_bi min_j T) / B

Sharding: one batch per NeuronCore (8 cores, SPMD). Each core emits
partial [128, 1]; the host sums and divides by B.

Single pass over the NxN matrix in 32 row-tiles of [128, 4096]. The PE
computes NEGATED distances U = -T = 2 p.g - ||g||^2 - ||p||^2 via one
augmented K=15 bf16 matmul per 512-chunk (2-level hi/mid coordinate
split with all 4 cross terms; 3-level split of -||g||^2 in the weights;
-||p||^2 added EXACTLY as a per-partition fp32 bias during the drain).
Mins become maxes on U.

Per-tile work split (balance ACT and DVE; period ~3.6us, DVE-bound):
  - ACT drains columns [0, A_COLS) of the fp32 PSUM to fp16 C with the
    -||p||^2 bias fused (activation Identity, bias AP): ~3.6us.
  - DVE drains the tail [A_COLS, N) via ONE tensor_scalar (op0=add bias,
    op1=max accum_out): drain + bias + row-max of the tail fused, 1x
    from PSUM (~0.35us).
  - DVE row-max of the ACT region: tensor_scalar op1=max accum_out over
    C[:, 0:A_COLS] — plain InstTensorScalarPtr runs in the DVE 4x_2p
    mode (0.26 ns/elem, everything fp16 in SBUF), replacing a
    5-instruction fold chain (~1.07us).
  - DVE column accumulator Mcol: one tensor_tensor max (2x mode,
    ~2.2us), with the first tile done as a tensor_copy so Mcol needs no
    memset. The row-max + Mcol ops are emitted with a one-tile lag so
    the PE's next matmuls only ever wait on the small TS-B drain.
A dummy PE transpose at the start of preprocessing starts the PE DVFS
ramp clock ~11us before the first real matmul (full 2.4GHz from tile 0;
without it the first tiles run at the 0.65/1.2GHz cold p-states).

Notes from the optimization campaign (TimelineSim is the graded clock):
  - Pool (gpsimd) compute ops and DMA cce min/max are rejected by this
    toolchain; gpsimd partition_all_reduce(max) DOES work on HW
    (~5.8us per [128,4096] tile) but its latency through the C-buffer
    ring kept stalling ACT/PE (PE p-state spirals), netting no gain.
  - tensor_tensor_reduce crashes at runtime; activation accum is
    add-only; is_scalar_tensor_tensor/scan disable the DVE fast modes.
  - fp16 matmul PSUM output is TRN3-only.

Endgame: 32 PE transposes of Mcol into fp16 PSUM (4 separate chunk
tiles so the 4 chunked reduces pipeline against the transposes),
reduce_max over the transposed axis, reduce_sum both directions,
negate, one [128,1] fp32 DMA out.
"""

import numpy as np

import concourse.bacc as bacc
import concourse.bass as bass
import concourse.bass_isa as bass_isa
import concourse.tile as tile
from concourse import mybir
from concourse.bass_utils import run_bass_kernel_spmd

N = 4096
D = 3
B = 8
P = 128            # SBUF/PSUM partitions
KP = N // P        # 32 points per partition in the staging layout
NT = N // P        # 32 row-tiles
CH = 512           # matmul moving free dim
HF = 2048          # PSUM half (4 banks of fp32)
KR = 15            # augmented contraction rows
# Tiles whose per-tile column-max runs on the otherwise idle Pool engine
# (gpsimd partition_all_reduce, ~5.8us each) instead of the DVE tensor_tensor
# accumulator (~2.25us). Starts only at tile 9 so the pipeline is fully warm
# before the long-latency Pool ops enter it (a cold-pipeline stall can reset
# the PE DVFS ramp and spiral).
POOL_TILES = frozenset()
NPOOL = len(POOL_TILES)
POOL_ROW = {it: r for r, it in enumerate(sorted(POOL_TILES))}
A_COLS1 = 3888     # ACT cols when the lagged heavy op is a DVE TensorTensor
A_COLS2 = 2784     # ACT cols when the lagged heavy op went to the Pool

f32 = mybir.dt.float32
f16 = mybir.dt.float16
bf16 = mybir.dt.bfloat16
MAX = mybir.AluOpType.max
ADD = mybir.AluOpType.add
MULT = mybir.AluOpType.mult
IDENT = mybir.ActivationFunctionType.Identity

TRACE = False
LAST_RESULT = None

_nc_cache = None


def _build_bass():
    nc = bacc.Bacc(
        "TRN2", target_bir_lowering=False, debug=False, num_devices=B,
        num_swdge_queues=4,
    )
    pred = nc.declare_dram_parameter("prediction", [N, D], f32, isOutput=False)
    gt = nc.declare_dram_parameter("ground_truth", [N, D], f32, isOutput=False)
    out_dram = nc.declare_dram_parameter("partial", [P, 1], f32, isOutput=True)

    with tile.TileContext(nc) as tc:
        with (
            tc.tile_pool(name="singles", bufs=1) as singles,
            tc.tile_pool(name="work", bufs=2) as work,
            tc.tile_pool(name="stage", bufs=8) as stage,
            tc.tile_pool(name="prpool", bufs=2) as prpool,
        ):
            # ---------- preprocessing ----------
            # transpose identity first (it is cheap and lets a dummy PE
            # transpose start the PE p-state ramp clock ~11us before the
            # first real matmul, which then runs at full clock)
            ones_t = singles.tile([P, P], f16, tag="ones_t")
            nc.vector.memset(ones_t, 1.0)
            ident = singles.tile([P, P], f16, tag="ident")
            nc.gpsimd.affine_select(
                out=ident, in_=ones_t, pattern=[[-1, P]],
                compare_op=mybir.AluOpType.is_equal, fill=0.0,
                base=0, channel_multiplier=1,
            )

            # stage both inputs immediately; the bias re-load of pred (yt,
            # many small descriptors) is emitted LAST so it cannot delay the
            # staging loads on the shared DMA engines
            xts = {}
            for nm, xdram in (("p", pred), ("g", gt)):
                xt = work.tile([P, KP, D], f32, tag=f"{nm}_xt")
                nc.sync.dma_start(
                    out=xt, in_=xdram[:].rearrange("(p k) d -> p k d", p=P)
                )
                xts[nm] = xt

            with tc.tile_pool(name="psum_warm", bufs=1, space="PSUM") as psw:
                warm = psw.tile([P, P], f16, tag="warm")
                nc.tensor.matmul(
                    warm, ident, ident, is_transpose=True, start=True, stop=True,
                )

            # Levels: 2-level bf16 split of the coordinates. Pred side is
            # scaled by +2 (exact in bf16).
            def levels(xdram, tag, scale2):
                xt = xts[tag]
                xr = work.tile([P, D, KP], f32, tag=f"{tag}_xr")
                nc.vector.tensor_copy(out=xr, in_=xt[:].rearrange("p k d -> p d k"))
                # negated squared norm first: the 3-level split of -||g||^2
                # and its flats are the longest preprocessing chain
                sq = work.tile([P, D, KP], f32, tag=f"{tag}_sq")
                nc.vector.tensor_mul(sq, xr, xr)
                n32 = work.tile([P, KP], f32, tag=f"{tag}_n32")
                nc.vector.tensor_add(n32, sq[:, 0, :], sq[:, 1, :])
                nc.vector.tensor_add(n32, n32, sq[:, 2, :])
                nneg = work.tile([P, KP], f32, tag=f"{tag}_nneg")
                nc.vector.tensor_scalar_mul(nneg, n32, -1.0)
                h16 = singles.tile([P, D, KP], bf16, tag=f"{tag}_h16")
                nc.vector.tensor_copy(out=h16, in_=xr)
                h32 = work.tile([P, D, KP], f32, tag=f"{tag}_h32")
                nc.vector.tensor_copy(out=h32, in_=h16)
                r1 = work.tile([P, D, KP], f32, tag=f"{tag}_r1")
                nc.vector.tensor_sub(r1, xr, h32)
                m16 = singles.tile([P, D, KP], bf16, tag=f"{tag}_m16")
                nc.vector.tensor_copy(out=m16, in_=r1)
                if scale2:
                    h2 = singles.tile([P, D, KP], bf16, tag=f"{tag}_h2")
                    nc.vector.tensor_scalar_mul(h2, h16, 2.0)
                    m2 = singles.tile([P, D, KP], bf16, tag=f"{tag}_m2")
                    nc.vector.tensor_scalar_mul(m2, m16, 2.0)
                    h16, m16 = h2, m2
                return h16, m16, nneg

            S_P = singles.tile([KR, N], bf16, tag="S_p")
            R_G = singles.tile([KR, N], bf16, tag="R_g")

            # ones rows for the norm contraction (memset on the idle Pool
            # engine; engine ops must start at partition 0, so DMA into
            # place — that DMA is emitted after the flats so it does not
            # head-block the scalar queue while the memset runs)
            ones3 = singles.tile([3, N], bf16, tag="ones3")
            nc.gpsimd.memset(ones3, 1.0)

            # "flat" DMAs: [128, 32] staging -> one 4096-wide row. The SWDGE
            # (gpsimd) queue is slower per DMA (~1040ns vs ~630ns) but runs
            # concurrently with the single shared HWDGE device, which also
            # carries the staging loads, dups and ones row; weight ~half the
            # flats onto SWDGE to balance the two queues.
            flat_engines = [nc.sync, nc.scalar, nc.gpsimd]
            flat_i = [0]

            def flat(dst, r, src2d):
                eng = flat_engines[flat_i[0] % len(flat_engines)]
                flat_i[0] += 1
                eng.dma_start(
                    out=dst[r : r + 1, :].rearrange("r (p k) -> r p k", p=P),
                    in_=src2d,
                )

            # pred levels + its flats and dup first so the S_P side of the
            # HWDGE queue drains while the gt levels compute
            ph2, pm2, _ = levels(pred, "p", scale2=True)
            for d in range(D):
                flat(S_P, 0 + d, ph2[:, d, :])
                flat(S_P, 6 + d, pm2[:, d, :])
            nc.sync.dma_start(out=S_P[3:6, :], in_=S_P[0:3, :])
            nc.scalar.dma_start(out=S_P[9:12, :], in_=S_P[6:9, :])

            gh, gm, ngneg = levels(gt, "g", scale2=False)
            for d in range(D):
                flat(R_G, 0 + d, gh[:, d, :])
                flat(R_G, 3 + d, gm[:, d, :])
            nc.sync.dma_start(out=R_G[6:9, :], in_=R_G[0:3, :])
            nc.scalar.dma_start(out=R_G[9:12, :], in_=R_G[3:6, :])

            # 3-level bf16 split of -||g||^2 -> NL [128, 3, KP]
            NL = singles.tile([P, 3, KP], bf16, tag="NL")
            nc.vector.tensor_copy(out=NL[:, 0, :], in_=ngneg)
            t32 = work.tile([P, KP], f32, tag="t32")
            nc.vector.tensor_copy(out=t32, in_=NL[:, 0, :])
            rr = work.tile([P, KP], f32, tag="rr")
            nc.vector.tensor_sub(rr, ngneg, t32)
            nc.vector.tensor_copy(out=NL[:, 1, :], in_=rr)
            nc.vector.tensor_copy(out=t32, in_=NL[:, 1, :])
            nc.vector.tensor_sub(rr, rr, t32)
            nc.vector.tensor_copy(out=NL[:, 2, :], in_=rr)
            for lv in range(3):
                flat(R_G, 12 + lv, NL[:, lv, :])
            nc.scalar.dma_start(out=S_P[12:15, :], in_=ones3)

            # -||p||^2 in bias layout npb[m, it] = -||pred[it*128+m]||^2:
            # second load of pred in row-tile-major order (many tiny
            # descriptors -> two chunks, emitted after the staging loads so
            # they cannot monopolize the shared DMA engines), norms
            # computed directly in that layout (fp32 exact).
            yt = work.tile([P, NT, D], f32, tag="yt")
            yv = pred[:].rearrange("(i m) d -> m i d", m=P)
            nc.scalar.dma_start(out=yt[:, 0 : NT // 2, :], in_=yv[:, 0 : NT // 2, :])
            nc.scalar.dma_start(out=yt[:, NT // 2 : NT, :], in_=yv[:, NT // 2 : NT, :])
            yr = work.tile([P, D, NT], f32, tag="yr")
            nc.vector.tensor_copy(out=yr, in_=yt[:].rearrange("m i d -> m d i"))
            ysq = work.tile([P, D, NT], f32, tag="ysq")
            nc.vector.tensor_mul(ysq, yr, yr)
            npb = singles.tile([P, NT], f32, tag="npb")
            nc.vector.tensor_add(npb, ysq[:, 0, :], ysq[:, 1, :])
            nc.vector.tensor_add(npb, npb, ysq[:, 2, :])
            nc.vector.tensor_scalar_mul(npb, npb, -1.0)

            Mcol = singles.tile([P, N], f16, tag="Mcol")
            Md2 = singles.tile([P, 2, NT], f32, tag="Md2")
            junk = singles.tile([P, N], f16, tag="junk")
            # per-pool-tile column maxes, one row per pool tile
            if NPOOL:
                stack = singles.tile([NPOOL, N], f16, tag="stack", name="stack")
            else:
                stack = None
            stack_row = 0
            # sync (SP) only: a scalar-engine DMA would occupy the ACT
            # sequencer for ~0.6us mid-loop and stall the drains
            stack_eng = [nc.sync, nc.sync]
            first_dve_tile = min(it for it in range(NT) if it not in POOL_TILES)

            # ---------- main loop ----------
            # The heavy per-tile DVE ops (full row-max, Mcol update) and the
            # Pool column-max are emitted with a ONE-TILE LAG: the DVE engine
            # is in-order, and the PE's next h2-matmuls wait on the small
            # TS-B drain, which must not queue behind a 2.2us TensorTensor.
            def acols(it):
                # tile it's drains share the DVE with tile it-1's lagged
                # heavy op; when that op runs on the Pool instead, the DVE
                # has room for a bigger TS-B drain share
                return A_COLS2 if (it - 1) in POOL_TILES else A_COLS1

            c_tiles = {}

            def heavy_ops(it):
                # row-max over the ACT region + column accumulator for tile it
                C = c_tiles.pop(it)
                a = acols(it)
                nc.vector.tensor_scalar(
                    out=junk[:, 0:a], in0=C[:, 0:a],
                    scalar1=1.0, scalar2=None, op0=MULT, op1=MAX,
                    accum_out=Md2[:, 0, it : it + 1],
                )
                if it in POOL_TILES:
                    PR = prpool.tile([P, N], f16, tag="PR")
                    nc.gpsimd.partition_all_reduce(
                        PR, C, channels=P, reduce_op=bass_isa.ReduceOp.max,
                    )
                    eng = stack_eng[POOL_ROW[it] % 2]
                    eng.dma_start(
                        out=stack[POOL_ROW[it] : POOL_ROW[it] + 1, :],
                        in_=PR[0:1, :],
                    )
                elif it == first_dve_tile:
                    nc.vector.tensor_copy(out=Mcol, in_=C)
                else:
                    nc.vector.tensor_tensor(Mcol, Mcol, C, MAX)

            with tc.tile_pool(name="psum_main", bufs=2, space="PSUM") as psum:
                for it in range(NT):
                    lhsT = S_P[0:KR, it * P : (it + 1) * P]
                    bias = npb[:, it : it + 1]
                    a = acols(it)
                    C = stage.tile([P, N], f16, tag="C")
                    c_tiles[it] = C
                    T = psum.tile([P, HF], f32, tag="T")
                    for q in range(4):
                        nc.tensor.matmul(
                            T[:, q * CH : (q + 1) * CH],
                            lhsT,
                            R_G[0:KR, q * CH : q * CH + CH],
                            start=True, stop=True,
                        )
                    nc.scalar.activation(
                        out=C[:, 0:HF], in_=T, func=IDENT, bias=bias, scale=1.0
                    )
                    T2 = psum.tile([P, HF], f32, tag="T")
                    for q in range(4):
                        c0 = HF + q * CH
                        nc.tensor.matmul(
                            T2[:, q * CH : (q + 1) * CH],
                            lhsT,
                            R_G[0:KR, c0 : c0 + CH],
                            start=True, stop=True,
                        )
                    nc.scalar.activation(
                        out=C[:, HF:a], in_=T2[:, 0 : a - HF],
                        func=IDENT, bias=bias, scale=1.0,
                    )
                    # fused drain + bias + row-max of the DVE tail region
                    nc.vector.tensor_scalar(
                        out=C[:, a:N], in0=T2[:, a - HF : HF],
                        scalar1=bias, scalar2=None, op0=ADD, op1=MAX,
                        accum_out=Md2[:, 1, it : it + 1],
                    )
                    if it > 0:
                        heavy_ops(it - 1)
                heavy_ops(NT - 1)

            # ---------- endgame ----------
            # colmax_j = max over 128 partitions of Mcol[:, j]: PE-transpose
            # the 32 [128,128] blocks into fp16 PSUM, fold over the
            # transposed axis, then sum. rowmax: merge Md2 halves, sum.
            with tc.tile_pool(name="psum_end", bufs=1, space="PSUM") as psum2:
                # PT_c[p, b, q] = Mcol[q, (c*CHK+b)*128+p]; max over q (last
                # axis). Separate PSUM tiles per chunk so the reduce of chunk
                # c has no WAR hazard against the transposes of chunk c+1.
                cmax = singles.tile([P, NT], f32, tag="cmax")
                CHK = 8
                W = P + NPOOL  # per block: 128 Mcol rows + NPOOL stack rows
                for c in range(NT // CHK):
                    PT = psum2.tile([P, CHK * W], f16, tag=f"PT{c}")
                    for b in range(CHK):
                        blk = c * CHK + b
                        nc.tensor.matmul(
                            PT[:, b * W : b * W + P],
                            Mcol[0:P, blk * P : (blk + 1) * P],
                            ident,
                            is_transpose=True,
                            start=True, stop=True,
                        )
                        if NPOOL:
                            nc.tensor.matmul(
                                PT[:, b * W + P : (b + 1) * W],
                                stack[0:NPOOL, blk * P : (blk + 1) * P],
                                ident[0:NPOOL, 0:NPOOL],
                                is_transpose=True,
                                start=True, stop=True,
                            )
                    nc.vector.tensor_reduce(
                        out=cmax[:, c * CHK : (c + 1) * CHK],
                        in_=PT[:].rearrange("p (b q) -> p b q", q=W),
                        axis=mybir.AxisListType.X, op=MAX,
                    )
                csum = singles.tile([P, 1], f32, tag="csum")
                nc.vector.reduce_sum(out=csum, in_=cmax, axis=mybir.AxisListType.X)
                Mdm = singles.tile([P, NT], f32, tag="Mdm")
                nc.vector.tensor_tensor(Mdm, Md2[:, 0, :], Md2[:, 1, :], MAX)
                rsum = singles.tile([P, 1], f32, tag="rsum")
                nc.vector.reduce_sum(out=rsum, in_=Mdm, axis=mybir.AxisListType.X)
                tot = singles.tile([P, 1], f32, tag="tot")
                nc.vector.tensor_add(tot, csum, rsum)
                nc.vector.tensor_scalar_mul(tot, tot, -1.0)
                nc.sync.dma_start(out=out_dram[:], in_=tot)

    nc.compile()
    return nc


def _get_nc():
    global _nc_cache
    if _nc_cache is None:
        _nc_cache = _build_bass()
    return _nc_cache


def kernel(prediction, ground_truth):
    global LAST_RESULT
    pred = np.ascontiguousarray(np.asarray(prediction, dtype=np.float32))
    gtr = np.ascontiguousarray(np.asarray(ground_truth, dtype=np.float32))
    assert pred.shape == (B, N, D) and gtr.shape == (B, N, D)
    nc = _get_nc()
    in_maps = [
        {"prediction": pred[b], "ground_truth": gtr[b]} for b in range(B)
    ]
    res = run_bass_kernel_spmd(nc, in_maps, list(range(B)), trace=TRACE)
    LAST_RESULT = res
    total = sum(float(np.sum(r["partial"], dtype=np.float64)) for r in res.results)
    return np.float32(total / B)


# revision 60
# speedup vs baseline: 1.0085x; 1.0085x over previous
"""Chamfer distance (CDLoss) Trainium2 Bass kernel.

Problem: B=8, N=4096, D=3.
  T[b,i,j] = ||pred[b,i] - gt[b,j]||^2
  loss = (sum_bj min_i T + sum_bi min_j T) / B

Sharding: one batch per NeuronCore (8 cores, SPMD). Each core emits
partial [128, 1]; the host sums and divides by B.

Single pass over the NxN matrix in 32 row-tiles of [128, 4096]. The PE
computes NEGATED distances U = -T = 2 p.g - ||g||^2 - ||p||^2 via one
augmented K=15 bf16 matmul per 512-chunk (2-level hi/mid coordinate
split with all 4 cross terms; 3-level split of -||g||^2 in the weights;
-||p||^2 added EXACTLY as a per-partition fp32 bias during the drain).
Mins become maxes on U.

Per-tile work split (balance ACT and DVE; period ~3.6us, DVE-bound):
  - ACT drains columns [0, A_COLS) of the fp32 PSUM to fp16 C with the
    -||p||^2 bias fused (activation Identity, bias AP): ~3.6us.
  - DVE drains the tail [A_COLS, N) via ONE tensor_scalar (op0=add bias,
    op1=max accum_out): drain + bias + row-max of the tail fused, 1x
    from PSUM (~0.35us).
  - DVE row-max of the ACT region: tensor_scalar op1=max accum_out over
    C[:, 0:A_COLS] — plain InstTensorScalarPtr runs in the DVE 4x_2p
    mode (0.26 ns/elem, everything fp16 in SBUF), replacing a
    5-instruction fold chain (~1.07us).
  - DVE column accumulator Mcol: one tensor_tensor max (2x mode,
    ~2.2us), with the first tile done as a tensor_copy so Mcol needs no
    memset. The row-max + Mcol ops are emitted with a one-tile lag so
    the PE's next matmuls only ever wait on the small TS-B drain.
A dummy PE transpose at the start of preprocessing starts the PE DVFS
ramp clock ~11us before the first real matmul (full 2.4GHz from tile 0;
without it the first tiles run at the 0.65/1.2GHz cold p-states).

Notes from the optimization campaign (TimelineSim is the graded clock):
  - Pool (gpsimd) compute ops and DMA cce min/max are rejected by this
    toolchain; gpsimd partition_all_reduce(max) DOES work on HW
    (~5.8us per [128,4096] tile) but its latency through the C-buffer
    ring kept stalling ACT/PE (PE p-state spirals), netting no gain.
  - tensor_tensor_reduce crashes at runtime; activation accum is
    add-only; is_scalar_tensor_tensor/scan disable the DVE fast modes.
  - fp16 matmul PSUM output is TRN3-only.

Endgame: 32 PE transposes of Mcol into fp16 PSUM (4 separate chunk
tiles so the 4 chunked reduces pipeline against the transposes),
reduce_max over the transposed axis, reduce_sum both directions,
negate, one [128,1] fp32 DMA out.
"""

import numpy as np

import concourse.bacc as bacc
import concourse.bass as bass
import concourse.bass_isa as bass_isa
import concourse.tile as tile
from concourse import mybir
from concourse.bass_utils import run_bass_kernel_spmd

N = 4096
D = 3
B = 8
P = 128            # SBUF/PSUM partitions
KP = N // P        # 32 points per partition in the staging layout
NT = N // P        # 32 row-tiles
CH = 512           # matmul moving free dim
HF = 2048          # PSUM half (4 banks of fp32)
KR = 15            # augmented contraction rows
# Tiles whose per-tile column-max runs on the otherwise idle Pool engine
# (gpsimd partition_all_reduce, ~5.8us each) instead of the DVE tensor_tensor
# accumulator (~2.25us). Starts only at tile 9 so the pipeline is fully warm
# before the long-latency Pool ops enter it (a cold-pipeline stall can reset
# the PE DVFS ramp and spiral).
POOL_TILES = frozenset()
NPOOL = len(POOL_TILES)
POOL_ROW = {it: r for r, it in enumerate(sorted(POOL_TILES))}
A_COLS1 = 3888     # ACT cols when the lagged heavy op is a DVE TensorTensor
A_COLS2 = 2784     # ACT cols when the lagged heavy op went to the Pool

f32 = mybir.dt.float32
f16 = mybir.dt.float16
bf16 = mybir.dt.bfloat16
MAX = mybir.AluOpType.max
ADD = mybir.AluOpType.add
MULT = mybir.AluOpType.mult
IDENT = mybir.ActivationFunctionType.Identity

TRACE = False
LAST_RESULT = None

_nc_cache = None


def _build_bass():
    nc = bacc.Bacc(
        "TRN2", target_bir_lowering=False, debug=False, num_devices=B,
        num_swdge_queues=4,
    )
    pred = nc.declare_dram_parameter("prediction", [N, D], f32, isOutput=False)
    gt = nc.declare_dram_parameter("ground_truth", [N, D], f32, isOutput=False)
    out_dram = nc.declare_dram_parameter("partial", [P, 1], f32, isOutput=True)

    with tile.TileContext(nc) as tc:
        with (
            tc.tile_pool(name="singles", bufs=1) as singles,
            tc.tile_pool(name="work", bufs=2) as work,
            tc.tile_pool(name="stage", bufs=8) as stage,
            tc.tile_pool(name="prpool", bufs=2) as prpool,
        ):
            # ---------- preprocessing ----------
            # transpose identity first (it is cheap and lets a dummy PE
            # transpose start the PE p-state ramp clock ~11us before the
            # first real matmul, which then runs at full clock)
            ones_t = singles.tile([P, P], f16, tag="ones_t")
            nc.vector.memset(ones_t, 1.0)
            ident = singles.tile([P, P], f16, tag="ident")
            nc.gpsimd.affine_select(
                out=ident, in_=ones_t, pattern=[[-1, P]],
                compare_op=mybir.AluOpType.is_equal, fill=0.0,
                base=0, channel_multiplier=1,
            )

            # stage both inputs immediately; the bias re-load of pred (yt,
            # many small descriptors) is emitted LAST so it cannot delay the
            # staging loads on the shared DMA engines
            xts = {}
            for nm, xdram in (("p", pred), ("g", gt)):
                xt = work.tile([P, KP, D], f32, tag=f"{nm}_xt")
                nc.sync.dma_start(
                    out=xt, in_=xdram[:].rearrange("(p k) d -> p k d", p=P)
                )
                xts[nm] = xt

            with tc.tile_pool(name="psum_warm", bufs=1, space="PSUM") as psw:
                warm = psw.tile([P, P], f16, tag="warm")
                nc.tensor.matmul(
                    warm, ident, ident, is_transpose=True, start=True, stop=True,
                )

            # Levels: 2-level bf16 split of the coordinates. Pred side is
            # scaled by +2 (exact in bf16).
            def levels(xdram, tag, scale2):
                xt = xts[tag]
                xr = work.tile([P, D, KP], f32, tag=f"{tag}_xr")
                nc.vector.tensor_copy(out=xr, in_=xt[:].rearrange("p k d -> p d k"))
                h16 = singles.tile([P, D, KP], bf16, tag=f"{tag}_h16")
                nc.vector.tensor_copy(out=h16, in_=xr)
                h32 = work.tile([P, D, KP], f32, tag=f"{tag}_h32")
                nc.vector.tensor_copy(out=h32, in_=h16)
                r1 = work.tile([P, D, KP], f32, tag=f"{tag}_r1")
                nc.vector.tensor_sub(r1, xr, h32)
                m16 = singles.tile([P, D, KP], bf16, tag=f"{tag}_m16")
                nc.vector.tensor_copy(out=m16, in_=r1)
                if scale2:
                    h2 = singles.tile([P, D, KP], bf16, tag=f"{tag}_h2")
                    nc.vector.tensor_scalar_mul(h2, h16, 2.0)
                    m2 = singles.tile([P, D, KP], bf16, tag=f"{tag}_m2")
                    nc.vector.tensor_scalar_mul(m2, m16, 2.0)
                    h16, m16 = h2, m2
                # negated squared norm (fp32)
                sq = work.tile([P, D, KP], f32, tag=f"{tag}_sq")
                nc.vector.tensor_mul(sq, xr, xr)
                n32 = work.tile([P, KP], f32, tag=f"{tag}_n32")
                nc.vector.tensor_add(n32, sq[:, 0, :], sq[:, 1, :])
                nc.vector.tensor_add(n32, n32, sq[:, 2, :])
                nneg = work.tile([P, KP], f32, tag=f"{tag}_nneg")
                nc.vector.tensor_scalar_mul(nneg, n32, -1.0)
                return h16, m16, nneg

            S_P = singles.tile([KR, N], bf16, tag="S_p")
            R_G = singles.tile([KR, N], bf16, tag="R_g")

            # ones rows for the norm contraction (memset on the idle Pool
            # engine; engine ops must start at partition 0, so DMA into
            # place — that DMA is emitted after the flats so it does not
            # head-block the scalar queue while the memset runs)
            ones3 = singles.tile([3, N], bf16, tag="ones3")
            nc.gpsimd.memset(ones3, 1.0)

            # "flat" DMAs: [128, 32] staging -> one 4096-wide row. The SWDGE
            # (gpsimd) queue is slower per DMA (~1040ns vs ~630ns) but runs
            # concurrently with the single shared HWDGE device, which also
            # carries the staging loads, dups and ones row; weight ~half the
            # flats onto SWDGE to balance the two queues.
            flat_engines = [nc.sync, nc.scalar, nc.gpsimd]
            flat_i = [0]

            def flat(dst, r, src2d):
                eng = flat_engines[flat_i[0] % len(flat_engines)]
                flat_i[0] += 1
                eng.dma_start(
                    out=dst[r : r + 1, :].rearrange("r (p k) -> r p k", p=P),
                    in_=src2d,
                )

            # pred levels + its flats and dup first so the S_P side of the
            # HWDGE queue drains while the gt levels compute
            ph2, pm2, _ = levels(pred, "p", scale2=True)
            for d in range(D):
                flat(S_P, 0 + d, ph2[:, d, :])
                flat(S_P, 6 + d, pm2[:, d, :])
            nc.sync.dma_start(out=S_P[3:6, :], in_=S_P[0:3, :])
            nc.scalar.dma_start(out=S_P[9:12, :], in_=S_P[6:9, :])

            gh, gm, ngneg = levels(gt, "g", scale2=False)
            for d in range(D):
                flat(R_G, 0 + d, gh[:, d, :])
                flat(R_G, 3 + d, gm[:, d, :])
            nc.sync.dma_start(out=R_G[6:9, :], in_=R_G[0:3, :])
            nc.scalar.dma_start(out=R_G[9:12, :], in_=R_G[3:6, :])

            # 3-level bf16 split of -||g||^2 -> NL [128, 3, KP]
            NL = singles.tile([P, 3, KP], bf16, tag="NL")
            nc.vector.tensor_copy(out=NL[:, 0, :], in_=ngneg)
            t32 = work.tile([P, KP], f32, tag="t32")
            nc.vector.tensor_copy(out=t32, in_=NL[:, 0, :])
            rr = work.tile([P, KP], f32, tag="rr")
            nc.vector.tensor_sub(rr, ngneg, t32)
            nc.vector.tensor_copy(out=NL[:, 1, :], in_=rr)
            nc.vector.tensor_copy(out=t32, in_=NL[:, 1, :])
            nc.vector.tensor_sub(rr, rr, t32)
            nc.vector.tensor_copy(out=NL[:, 2, :], in_=rr)
            for lv in range(3):
                flat(R_G, 12 + lv, NL[:, lv, :])
            nc.scalar.dma_start(out=S_P[12:15, :], in_=ones3)

            # -||p||^2 in bias layout npb[m, it] = -||pred[it*128+m]||^2:
            # second load of pred in row-tile-major order (many tiny
            # descriptors -> two chunks, emitted after the staging loads so
            # they cannot monopolize the shared DMA engines), norms
            # computed directly in that layout (fp32 exact).
            yt = work.tile([P, NT, D], f32, tag="yt")
            yv = pred[:].rearrange("(i m) d -> m i d", m=P)
            nc.scalar.dma_start(out=yt[:, 0 : NT // 2, :], in_=yv[:, 0 : NT // 2, :])
            nc.scalar.dma_start(out=yt[:, NT // 2 : NT, :], in_=yv[:, NT // 2 : NT, :])
            yr = work.tile([P, D, NT], f32, tag="yr")
            nc.vector.tensor_copy(out=yr, in_=yt[:].rearrange("m i d -> m d i"))
            ysq = work.tile([P, D, NT], f32, tag="ysq")
            nc.vector.tensor_mul(ysq, yr, yr)
            npb = singles.tile([P, NT], f32, tag="npb")
            nc.vector.tensor_add(npb, ysq[:, 0, :], ysq[:, 1, :])
            nc.vector.tensor_add(npb, npb, ysq[:, 2, :])
            nc.vector.tensor_scalar_mul(npb, npb, -1.0)

            Mcol = singles.tile([P, N], f16, tag="Mcol")
            Md2 = singles.tile([P, 2, NT], f32, tag="Md2")
            junk = singles.tile([P, N], f16, tag="junk")
            # per-pool-tile column maxes, one row per pool tile
            if NPOOL:
                stack = singles.tile([NPOOL, N], f16, tag="stack", name="stack")
            else:
                stack = None
            stack_row = 0
            # sync (SP) only: a scalar-engine DMA would occupy the ACT
            # sequencer for ~0.6us mid-loop and stall the drains
            stack_eng = [nc.sync, nc.sync]
            first_dve_tile = min(it for it in range(NT) if it not in POOL_TILES)

            # ---------- main loop ----------
            # The heavy per-tile DVE ops (full row-max, Mcol update) and the
            # Pool column-max are emitted with a ONE-TILE LAG: the DVE engine
            # is in-order, and the PE's next h2-matmuls wait on the small
            # TS-B drain, which must not queue behind a 2.2us TensorTensor.
            def acols(it):
                # tile it's drains share the DVE with tile it-1's lagged
                # heavy op; when that op runs on the Pool instead, the DVE
                # has room for a bigger TS-B drain share
                return A_COLS2 if (it - 1) in POOL_TILES else A_COLS1

            c_tiles = {}

            def heavy_ops(it):
                # row-max over the ACT region + column accumulator for tile it
                C = c_tiles.pop(it)
                a = acols(it)
                nc.vector.tensor_scalar(
                    out=junk[:, 0:a], in0=C[:, 0:a],
                    scalar1=1.0, scalar2=None, op0=MULT, op1=MAX,
                    accum_out=Md2[:, 0, it : it + 1],
                )
                if it in POOL_TILES:
                    PR = prpool.tile([P, N], f16, tag="PR")
                    nc.gpsimd.partition_all_reduce(
                        PR, C, channels=P, reduce_op=bass_isa.ReduceOp.max,
                    )
                    eng = stack_eng[POOL_ROW[it] % 2]
                    eng.dma_start(
                        out=stack[POOL_ROW[it] : POOL_ROW[it] + 1, :],
                        in_=PR[0:1, :],
                    )
                elif it == first_dve_tile:
                    nc.vector.tensor_copy(out=Mcol, in_=C)
                else:
                    nc.vector.tensor_tensor(Mcol, Mcol, C, MAX)

            with tc.tile_pool(name="psum_main", bufs=2, space="PSUM") as psum:
                for it in range(NT):
                    lhsT = S_P[0:KR, it * P : (it + 1) * P]
                    bias = npb[:, it : it + 1]
                    a = acols(it)
                    C = stage.tile([P, N], f16, tag="C")
                    c_tiles[it] = C
                    T = psum.tile([P, HF], f32, tag="T")
                    for q in range(4):
                        nc.tensor.matmul(
                            T[:, q * CH : (q + 1) * CH],
                            lhsT,
                            R_G[0:KR, q * CH : q * CH + CH],
                            start=True, stop=True,
                        )
                    nc.scalar.activation(
                        out=C[:, 0:HF], in_=T, func=IDENT, bias=bias, scale=1.0
                    )
                    T2 = psum.tile([P, HF], f32, tag="T")
                    for q in range(4):
                        c0 = HF + q * CH
                        nc.tensor.matmul(
                            T2[:, q * CH : (q + 1) * CH],
                            lhsT,
                            R_G[0:KR, c0 : c0 + CH],
                            start=True, stop=True,
                        )
                    nc.scalar.activation(
                        out=C[:, HF:a], in_=T2[:, 0 : a - HF],
                        func=IDENT, bias=bias, scale=1.0,
                    )
                    # fused drain + bias + row-max of the DVE tail region
                    nc.vector.tensor_scalar(
                        out=C[:, a:N], in0=T2[:, a - HF : HF],
                        scalar1=bias, scalar2=None, op0=ADD, op1=MAX,
                        accum_out=Md2[:, 1, it : it + 1],
                    )
                    if it > 0:
                        heavy_ops(it - 1)
                heavy_ops(NT - 1)

            # ---------- endgame ----------
            # colmax_j = max over 128 partitions of Mcol[:, j]: PE-transpose
            # the 32 [128,128] blocks into fp16 PSUM, fold over the
            # transposed axis, then sum. rowmax: merge Md2 halves, sum.
            with tc.tile_pool(name="psum_end", bufs=1, space="PSUM") as psum2:
                # PT_c[p, b, q] = Mcol[q, (c*CHK+b)*128+p]; max over q (last
                # axis). Separate PSUM tiles per chunk so the reduce of chunk
                # c has no WAR hazard against the transposes of chunk c+1.
                cmax = singles.tile([P, NT], f32, tag="cmax")
                CHK = 8
                W = P + NPOOL  # per block: 128 Mcol rows + NPOOL stack rows
                for c in range(NT // CHK):
                    PT = psum2.tile([P, CHK * W], f16, tag=f"PT{c}")
                    for b in range(CHK):
                        blk = c * CHK + b
                        nc.tensor.matmul(
                            PT[:, b * W : b * W + P],
                            Mcol[0:P, blk * P : (blk + 1) * P],
                            ident,
                            is_transpose=True,
                            start=True, stop=True,
                        )
                        if NPOOL:
                            nc.tensor.matmul(
                                PT[:, b * W + P : (b + 1) * W],
                                stack[0:NPOOL, blk * P : (blk + 1) * P],
                                ident[0:NPOOL, 0:NPOOL],
                                is_transpose=True,
                                start=True, stop=True,
                            )
                    nc.vector.tensor_reduce(
                        out=cmax[:, c * CHK : (c + 1) * CHK],
                        in_=PT[:].rearrange("p (b q) -> p b q", q=W),
                        axis=mybir.AxisListType.X, op=MAX,
                    )
                csum = singles.tile([P, 1], f32, tag="csum")
                nc.vector.reduce_sum(out=csum, in_=cmax, axis=mybir.AxisListType.X)
                Mdm = singles.tile([P, NT], f32, tag="Mdm")
                nc.vector.tensor_tensor(Mdm, Md2[:, 0, :], Md2[:, 1, :], MAX)
                rsum = singles.tile([P, 1], f32, tag="rsum")
                nc.vector.reduce_sum(out=rsum, in_=Mdm, axis=mybir.AxisListType.X)
                tot = singles.tile([P, 1], f32, tag="tot")
                nc.vector.tensor_add(tot, csum, rsum)
                nc.vector.tensor_scalar_mul(tot, tot, -1.0)
                nc.sync.dma_start(out=out_dram[:], in_=tot)

    nc.compile()
    return nc


def _get_nc():
    global _nc_cache
    if _nc_cache is None:
        _nc_cache = _build_bass()
    return _nc_cache


def kernel(prediction, ground_truth):
    global LAST_RESULT
    pred = np.ascontiguousarray(np.asarray(prediction, dtype=np.float32))
    gtr = np.ascontiguousarray(np.asarray(ground_truth, dtype=np.float32))
    assert pred.shape == (B, N, D) and gtr.shape == (B, N, D)
    nc = _get_nc()
    in_maps = [
        {"prediction": pred[b], "ground_truth": gtr[b]} for b in range(B)
    ]
    res = run_bass_kernel_spmd(nc, in_maps, list(range(B)), trace=TRACE)
    LAST_RESULT = res
    total = sum(float(np.sum(r["partial"], dtype=np.float64)) for r in res.results)
    return np.float32(total / B)
